# revision 10
# baseline (speedup 1.0000x reference)
"""Trainium2 Bass kernel for nn_CSSMSHViT_60043642798201.

Strategy
--------
The reference repeats the input image over a time axis T=8 and runs a gated
scalar recurrence over T.  Because the input is constant over T the whole
temporal structure collapses algebraically:

    h_t = (1 - a^{t+1}) z          (closed form of the scan)

so the per-timestep fields are never materialised.  The kernel computes

  LN1 (global per-batch) -> +3x3 depthwise pos conv -> z/sigma/g projections
  -> power ladder u_t = a^t z with fused per-batch reductions St = sum(u_t*Gt)
     where Gt = DW5^T(g)  (adjoint trick: mean(DW5(u)*g) = mean(u*DW5^T(g)))
  -> tiny gate MLP -> softmax weights w
  -> F = z - sum_t w_t u_{t+1};  x_out = (DW5(F)+b_sp)*g @ W_out + b_out
  -> out1 = x + x_out -> LN2 -> MLP with 3x3 depthwise conv -> out.

Sharding: pure data-parallel over batch (32 = 8 cores x 4), no collectives.

Layouts: channel-major [C_partition, (b, h, w)] SBUF fields (the host does
the NHWC <-> channel-major transposes); depthwise convs run on the
TensorEngine as diagonal-weight matmuls accumulated over taps in PSUM with
shifted access patterns into zero-padded buffers.  Dense projections run in
fp8 (weights pre-scaled x256 on the host, rescaled in the PSUM evacuation)
using DoubleRow perf mode to contract two 128-channel chunks per pass.  The
LN1 affine is commuted through the (linear) positional conv so the conv runs
on raw x and needn't wait for LN1 statistics.  The middle section is
pipelined over two batch-halves, the power ladder fuses its per-batch
reductions into scalar_tensor_tensor accum outputs, and the third channel
chunk of the ladder/Horner recurrences runs on the (otherwise idle) GpSimd
engine.
"""

import numpy as np
import ml_dtypes

BF16 = ml_dtypes.bfloat16
FP8 = ml_dtypes.float8_e4m3

# problem constants
B, T, H, W, C = 32, 8, 16, 16, 384
KS = 5
HID = 4 * C
GH = max(C // 4, 8)
RHO = 0.999
EPS = 1e-6

NCORES = 8
BL = B // NCORES            # batches per core = 4
HWN = H * W                 # 256 tokens per image
NTOK = BL * HWN             # 1024 tokens per core
NCC = C // 128              # 3 channel chunks
NHC = HID // 128            # 12 hidden chunks

WS = 256.0                  # fp8 weight pre-scale (avoids subnormals)
XOS = 16.0                  # F/x_out field pre-scale for fp8 range

# padded geometries (channel-major fields, free layout (b, hp, wp))
H1, W1P = 18, 18            # pad-1 buffers (3x3 convs)
F1 = BL * H1 * W1P
H2, W2P = 20, 20            # pad-2 buffers (5x5 convs)
F2 = BL * H2 * W2P

_PROG = None  # cached compiled program


def _build_program():
    import concourse.bass as bass
    import concourse.tile as tile
    from concourse import bacc, mybir

    fp32 = mybir.dt.float32
    bf16 = mybir.dt.bfloat16
    f8 = mybir.dt.float8e4
    AF = mybir.ActivationFunctionType
    OP = mybir.AluOpType
    AX = mybir.AxisListType
    DR = mybir.MatmulPerfMode.DoubleRow

    nc = bacc.Bacc("TRN2", target_bir_lowering=False)

    # ---------------- DRAM tensors ----------------
    d = {}
    d["x_cm"] = nc.dram_tensor("x_cm", [128, NCC, NTOK], bf16, kind="ExternalInput")
    d["x_pad"] = nc.dram_tensor("x_pad", [128, NCC, F1], bf16, kind="ExternalInput")
    d["sp1"] = nc.dram_tensor("sp1", [128, NCC, HWN], bf16, kind="ExternalInput")
    # fp8 matmul weights (pre-scaled x256), chunked [128, kchunks, M]
    d["w_in8"] = nc.dram_tensor("w_in8", [128, NCC, C], f8, kind="ExternalInput")
    d["w_a8"] = nc.dram_tensor("w_a8", [128, NCC, C], f8, kind="ExternalInput")
    d["w_g8"] = nc.dram_tensor("w_g8", [128, NCC, C], f8, kind="ExternalInput")
    d["w_out8"] = nc.dram_tensor("w_out8", [128, NCC, C], f8, kind="ExternalInput")
    d["w1_8"] = nc.dram_tensor("w1_8", [128, NCC, HID], f8, kind="ExternalInput")
    d["w2_8"] = nc.dram_tensor("w2_8", [128, NHC, C], f8, kind="ExternalInput")
    d["w_out"] = nc.dram_tensor("w_out", [128, NCC, C], bf16, kind="ExternalInput")
    d["wg1"] = nc.dram_tensor("wg1", [128, 2 * NCC, GH], bf16, kind="ExternalInput")
    d["wg2"] = nc.dram_tensor("wg2", [GH, 1], bf16, kind="ExternalInput")
    # diagonalised depthwise kernels (partition-major, contiguous per partition)
    d["dpos"] = nc.dram_tensor("dpos", [128, NCC, 9, 128], bf16, kind="ExternalInput")
    d["dsp"] = nc.dram_tensor("dsp", [128, NCC, 25, 128], bf16, kind="ExternalInput")
    d["ddw"] = nc.dram_tensor("ddw", [128, NHC, 9, 128], f8, kind="ExternalInput")
    # per-channel vectors [128, nchunks] fp32
    for nm in ["b_in", "b_a", "b_g", "b_sp", "b_sp16", "b_out", "b2",
               "gamma1", "beta1", "b_pos"]:
        d[nm] = nc.dram_tensor(nm, [128, NCC], fp32, kind="ExternalInput")
    d["b1"] = nc.dram_tensor("b1", [128, NHC], fp32, kind="ExternalInput")
    d["bdw"] = nc.dram_tensor("bdw", [128, NHC], fp32, kind="ExternalInput")
    d["g2r"] = nc.dram_tensor("g2r", [1, NCC, 128], bf16, kind="ExternalInput")
    d["be2"] = nc.dram_tensor("be2", [128, NCC], fp32, kind="ExternalInput")
    d["bg1"] = nc.dram_tensor("bg1", [GH, 1], fp32, kind="ExternalInput")
    d["bg2"] = nc.dram_tensor("bg2", [1, 1], fp32, kind="ExternalInput")
    d["prior"] = nc.dram_tensor("prior", [1, BL * T], fp32, kind="ExternalInput")
    out_d = nc.dram_tensor("out", [128, NCC, NTOK], fp32, kind="ExternalOutput")

    with tile.TileContext(nc) as tc:
        _emit(nc, tc, d, out_d, mybir, bass, fp32, bf16, f8, AF, OP, AX, DR)

    nc.compile()
    return nc


def _emit(nc, tc, d, out_d, mybir, bass, fp32, bf16, f8, AF, OP, AX, DR):
    from contextlib import ExitStack
    ctx = ExitStack()

    pool = ctx.enter_context(tc.tile_pool(name="persist", bufs=1))
    dpool = ctx.enter_context(tc.tile_pool(name="ddwst", bufs=3))
    scr = ctx.enter_context(tc.tile_pool(name="scratch", bufs=2))
    pp_mm = ctx.enter_context(tc.tile_pool(name="pp_mm", bufs=6, space="PSUM"))
    pp_sm = ctx.enter_context(tc.tile_pool(name="pp_sm", bufs=1, space="PSUM"))

    import concourse.bass_isa as bass_isa
    RADD = bass_isa.ReduceOp.add
    SCL = 1.0 / WS

    # ---------------- persistent field tiles ----------------
    x_cm = pool.tile([128, NCC, NTOK], bf16, name="x_cm")
    xn0p = pool.tile([128, NCC, F1], bf16, name="xn0p")            # padded raw x
    sp1 = pool.tile([128, NCC, HWN], bf16, name="sp1")             # 1 + conv3(1)
    xpos = pool.tile([128, NCC, NTOK], f8, name="xpos")            # also xo_rhs
    z_f = pool.tile([128, NCC, NTOK], bf16, name="z_f")            # reused LN2 tmp
    sg_f = pool.tile([128, NCC, NTOK], bf16, name="sg_f")
    g_p = pool.tile([128, NCC, F2], bf16, name="g_p")              # padded silu gate
    gt_f = pool.tile([128, NCC, NTOK], bf16, name="gt_f")          # Gt; later a=rho*sig
    u_f = pool.tile([128, NCC, NTOK], bf16, name="u_f")            # ladder A/acc/sq
    q2 = pool.tile([128, NCC, NTOK], bf16, name="q2")              # ladder B / o1b
    f_p = pool.tile([128, NCC, F2], bf16, name="f_p")              # padded XOS*F field
    out1 = pool.tile([128, NCC, NTOK], fp32, name="out1")          # also final out
    yn8 = pool.tile([128, NCC, NTOK], f8, name="yn8")
    h1p = pool.tile([128, NHC, F1], f8, name="h1p")                # padded MLP hidden
    h1g = pool.tile([128, NHC, NTOK], f8, name="h1g")              # dense gelu(conv)

    # weights
    w_in8t = pool.tile([128, NCC, C], f8, name="w_in8t")
    w_a8t = pool.tile([128, NCC, C], f8, name="w_a8t")
    w_g8t = pool.tile([128, NCC, C], f8, name="w_g8t")
    w_out8t = pool.tile([128, NCC, C], f8, name="w_out8t")
    w1_8t = pool.tile([128, NCC, HID], f8, name="w1_8t")
    w2_8t = pool.tile([128, NHC, C], f8, name="w2_8t")
    w_out_t = pool.tile([128, NCC, C], bf16, name="w_out_t")       # for gate k path
    wg1_t = pool.tile([128, 2 * NCC, GH], bf16, name="wg1_t")
    wg2_t = pool.tile([GH, 1], bf16, name="wg2_t")
    dsp_t = pool.tile([128, NCC, 25, 128], bf16, name="dsp_t")     # resident 5x5 diags
    dpos_t = pool.tile([128, NCC, 9, 128], bf16, name="dpos_t")    # resident 3x3 diags

    # vectors
    b_in_c = pool.tile([128, NCC], fp32, name="b_in_c")
    b_a_c = pool.tile([128, NCC], fp32, name="b_a_c")
    b_g_c = pool.tile([128, NCC], fp32, name="b_g_c")
    b_sp_c = pool.tile([128, NCC], fp32, name="b_sp_c")
    b_sp16_c = pool.tile([128, NCC], fp32, name="b_sp16_c")
    b_pos_c = pool.tile([128, NCC], fp32, name="b_pos_c")
    b_out_c = pool.tile([128, NCC], fp32, name="b_out_c")
    b2_c = pool.tile([128, NCC], fp32, name="b2_c")
    g1_c = pool.tile([128, NCC], fp32, name="g1_c")
    be1_c = pool.tile([128, NCC], fp32, name="be1_c")
    b1_c = pool.tile([128, NHC], fp32, name="b1_c")
    bdw_c = pool.tile([128, NHC], fp32, name="bdw_c")
    g2_t = pool.tile([1, NCC, 128], bf16, name="g2_t")
    be2_c = pool.tile([128, NCC], fp32, name="be2_c")
    bg1_c = pool.tile([GH, 1], fp32, name="bg1_c")
    bg2_c = pool.tile([1, 1], fp32, name="bg2_c")
    prior_r = pool.tile([1, BL * T], fp32, name="prior_r")

    # small working tiles
    ones_c = pool.tile([128, 1], bf16, name="ones_c")
    sums = pool.tile([128, 24], fp32, name="sums")       # stat*12 + b*3 + kc
    ar = pool.tile([128, 24], fp32, name="ar")
    tot = pool.tile([128, 2, BL], fp32, name="tot")
    m_col = pool.tile([128, BL], fp32, name="m_col")
    e2_col = pool.tile([128, BL], fp32, name="e2_col")
    var_col = pool.tile([128, BL], fp32, name="var_col")
    rstd_col = pool.tile([128, BL], fp32, name="rstd_col")
    sc_col = pool.tile([128, NCC, BL], fp32, name="sc_col")
    bi_col = pool.tile([128, NCC, BL], fp32, name="bi_col")
    tmp_col = pool.tile([128, BL], fp32, name="tmp_col")
    st_all = pool.tile([128, NCC, BL, T], fp32, name="st_all")
    s0_c = pool.tile([128, NCC, BL], fp32, name="s0_c")
    gbar_c = pool.tile([128, NCC, BL], fp32, name="gbar_c")
    s0gb = pool.tile([128, NCC, BL], fp32, name="s0gb")
    kv = pool.tile([128, NCC, BL, T], bf16, name="kv")
    qt = pool.tile([128, NCC, BL, T], bf16, name="qt")
    kw = pool.tile([128, NCC, BL * T], bf16, name="kw")
    hg = pool.tile([GH, BL * T], bf16, name="hg")
    logits = pool.tile([1, BL * T], fp32, name="logits")
    mx_r = pool.tile([1, BL], fp32, name="mx_r")
    esh = pool.tile([1, BL * T], fp32, name="esh")
    se_r = pool.tile([1, BL], fp32, name="se_r")
    wneg = pool.tile([1, BL * T], fp32, name="wneg")
    wbc = pool.tile([128, BL * T], fp32, name="wbc")
    stats2 = pool.tile([1, 2, NTOK], fp32, name="stats2")   # LN2 sums
    work2 = pool.tile([1, NTOK], fp32, name="work2")
    lnv2 = pool.tile([1, NTOK], fp32, name="lnv2")
    rhsS = pool.tile([1, NTOK], bf16, name="rhsS")          # rstd
    rhsM = pool.tile([1, NTOK], bf16, name="rhsM")          # -mu*rstd

    # ---------------- loads (conv operands first; then consumption order) ----
    for kc in range(NCC):
        nc.sync.dma_start(xn0p[:, kc, :], d["x_pad"][:, kc, :])
    for kc in range(NCC):
        nc.sync.dma_start(dpos_t[:, kc], d["dpos"][:, kc])
    for kc in range(NCC):
        nc.sync.dma_start(x_cm[:, kc, :], d["x_cm"][:, kc, :])
    nc.sync.dma_start(sp1[:], d["sp1"][:])

    def ld(tile_ap, dram):
        nc.sync.dma_start(tile_ap[:], dram[:])

    for nm, t_ in [("gamma1", g1_c), ("beta1", be1_c), ("b_pos", b_pos_c),
                   ("b_in", b_in_c), ("b_a", b_a_c), ("b_g", b_g_c),
                   ("b_sp", b_sp_c), ("b_sp16", b_sp16_c),
                   ("b_out", b_out_c), ("b2", b2_c)]:
        ld(t_, d[nm])
    ld(w_g8t, d["w_g8"])
    ld(w_in8t, d["w_in8"])
    ld(w_a8t, d["w_a8"])
    for kc in range(NCC):
        nc.sync.dma_start(dsp_t[:, kc], d["dsp"][:, kc])
    ld(w_out_t, d["w_out"])
    ld(w_out8t, d["w_out8"])
    ld(wg1_t, d["wg1"])
    nc.sync.dma_start(wg2_t[:], d["wg2"][:])
    nc.sync.dma_start(g2_t[:], d["g2r"][:])
    ld(be2_c, d["be2"])
    nc.sync.dma_start(bg1_c[:], d["bg1"][:])
    nc.sync.dma_start(bg2_c[:], d["bg2"][:])
    nc.sync.dma_start(prior_r[:], d["prior"][:])
    ld(w1_8t, d["w1_8"])
    ld(w2_8t, d["w2_8"])
    ld(b1_c, d["b1"])
    ld(bdw_c, d["bdw"])

    nc.vector.memset(ones_c[:], 1.0)

    # zero padded buffers (borders must stay zero)
    nc.gpsimd.memset(g_p[:].rearrange("p a b -> p (a b)"), 0.0)
    nc.gpsimd.memset(f_p[:].rearrange("p a b -> p (a b)"), 0.0)
    nc.gpsimd.memset(h1p[:].rearrange("p a b -> p (a b)"), 0.0)

    # view helpers -------------------------------------------------
    def pad1(tile_, j):           # -> [128, BL, H1, W1P] for chunk j
        return tile_[:, j, :].rearrange("p (b h w) -> p b h w", b=BL, h=H1, w=W1P)

    def pad2(tile_, j):
        return tile_[:, j, :].rearrange("p (b h w) -> p b h w", b=BL, h=H2, w=W2P)

    def dense(tile_, j):          # -> [128, BL, H, W]
        return tile_[:, j, :].rearrange("p (b h w) -> p b h w", b=BL, h=H, w=W)

    def int1(tile_, j):           # pad1 interior
        return pad1(tile_, j)[:, :, 1:1 + H, 1:1 + W]

    def int2(tile_, j):
        return pad2(tile_, j)[:, :, 2:2 + H, 2:2 + W]

    HV = NTOK // 512              # 2 halves (2 batches each)

    # ---------------- B: LN1 stats ----------------
    sview = sums[:].rearrange("p (s b k) -> p s b k", s=2, b=BL, k=NCC)
    for kc in range(NCC):
        nc.vector.tensor_reduce(
            sview[:, 0, :, kc],
            x_cm[:, kc, :].rearrange("p (b n) -> p b n", b=BL),
            axis=AX.X, op=OP.add)
        for b in range(BL):
            s_sc = scr.tile([128, HWN], bf16, tag="st_scr", name=f"sxx{kc}{b}")
            nc.scalar.activation(
                s_sc[:], x_cm[:, kc, b * HWN:(b + 1) * HWN], AF.Square,
                accum_out=sview[:, 1, b, kc:kc + 1])
    nc.gpsimd.partition_all_reduce(ar[:], sums[:], channels=128, reduce_op=RADD)
    nc.vector.tensor_reduce(
        tot[:], ar[:].rearrange("p (s b k) -> p s b k", s=2, b=BL, k=NCC),
        axis=AX.X, op=OP.add)
    NB = float(HWN * C)
    nc.vector.tensor_scalar(m_col[:], tot[:, 0, :], 1.0 / NB, None, op0=OP.mult)
    nc.vector.tensor_scalar(e2_col[:], tot[:, 1, :], 1.0 / NB, None, op0=OP.mult)
    nc.vector.tensor_tensor(tmp_col[:], m_col[:], m_col[:], op=OP.mult)
    nc.vector.tensor_tensor(var_col[:], e2_col[:], tmp_col[:], op=OP.subtract)
    nc.vector.tensor_scalar(var_col[:], var_col[:], EPS, None, op0=OP.add)
    nc.scalar.sqrt(var_col[:], var_col[:])
    nc.vector.reciprocal(rstd_col[:], var_col[:])
    for kc in range(NCC):
        nc.vector.tensor_scalar(
            sc_col[:, kc, :], rstd_col[:], g1_c[:, kc:kc + 1], None, op0=OP.mult)
        nc.vector.tensor_tensor(tmp_col[:], m_col[:], sc_col[:, kc, :], op=OP.mult)
        nc.vector.tensor_scalar(
            bi_col[:, kc, :], tmp_col[:], be1_c[:, kc:kc + 1], -1.0,
            op0=OP.subtract, op1=OP.mult)

    # ---------------- C: positional conv on raw x (identity-augmented) -------
    # xpos = sc*(x + conv3(x)) + bi*(1 + conv3(1)) + b_pos   (LN1 affine
    # commuted through the linear conv; dpos has +1 on the centre tap).
    for kc in range(NCC):
        for hv in range(HV):
            ps = pp_mm.tile([128, 512], fp32, tag="mm", name=f"cpos{kc}{hv}")
            for ti, (i, j) in enumerate([(a, b) for a in range(3) for b in range(3)]):
                rhs = pad1(xn0p, kc)[:, 2 * hv:2 * hv + 2, i:i + H, j:j + W]
                nc.tensor.matmul(
                    ps[:], dpos_t[:, kc, ti, :], rhs,
                    start=(ti == 0), stop=(ti == 8))
            ps4 = ps[:].rearrange("p (b h w) -> p b h w", b=2, h=H, w=W)
            for bb in range(2):
                b = 2 * hv + bb
                bia = scr.tile([128, HWN], bf16, tag="bia", name=f"bia{kc}{b}")
                nc.vector.tensor_scalar(
                    bia[:], sp1[:, kc, :], bi_col[:, kc, b:b + 1],
                    b_pos_c[:, kc:kc + 1], op0=OP.mult, op1=OP.add)
                nc.vector.scalar_tensor_tensor(
                    dense(xpos, kc)[:, b], ps4[:, bb], sc_col[:, kc, b:b + 1],
                    bia[:].rearrange("p (h w) -> p h w", h=H),
                    op0=OP.mult, op1=OP.add)

    # ---------------- D: z / sigma / g projections (fp8 DoubleRow) ----------
    def mm_c(dst_evac, w8t):
        for mc in range(NCC):
            for hv in range(HV):
                ps = pp_mm.tile([128, 512], fp32, tag="mm",
                                name=f"mmc_{id(w8t)}_{mc}_{hv}")
                nc.tensor.matmul(
                    ps[:], w8t[:, 0:2, mc * 128:(mc + 1) * 128],
                    xpos[:, 0:2, hv * 512:(hv + 1) * 512],
                    start=True, stop=False, perf_mode=DR)
                nc.tensor.matmul(
                    ps[:], w8t[:, 2, mc * 128:(mc + 1) * 128],
                    xpos[:, 2, hv * 512:(hv + 1) * 512],
                    start=False, stop=True)
                dst_evac(mc, hv, ps)

    def evac_z(mc, hv, ps):
        nc.scalar.activation(z_f[:, mc, hv * 512:(hv + 1) * 512], ps[:],
                             AF.Identity, bias=b_in_c[:, mc:mc + 1], scale=SCL)

    def evac_sg(mc, hv, ps):
        nc.scalar.activation(sg_f[:, mc, hv * 512:(hv + 1) * 512], ps[:],
                             AF.Sigmoid, bias=b_a_c[:, mc:mc + 1], scale=SCL)

    def evac_g(mc, hv, ps):
        # silu(v) = v * sigmoid(v), v = psum/WS + b_g  (no Silu LUT on trn2)
        ps4 = ps[:].rearrange("p (b h w) -> p b h w", b=2, h=H, w=W)
        vt = scr.tile([128, 512], bf16, tag="gv", name=f"gv{mc}{hv}")
        nc.scalar.activation(vt[:], ps[:], AF.Identity,
                             bias=b_g_c[:, mc:mc + 1], scale=SCL)
        vt4 = vt[:].rearrange("p (b h w) -> p b h w", b=2, h=H, w=W)
        for bb in range(2):
            nc.scalar.activation(
                pad2(g_p, mc)[:, 2 * hv + bb, 2:2 + H, 2:2 + W], ps4[:, bb],
                AF.Sigmoid, bias=b_g_c[:, mc:mc + 1], scale=SCL)
            nc.vector.tensor_tensor(
                pad2(g_p, mc)[:, 2 * hv + bb, 2:2 + H, 2:2 + W],
                pad2(g_p, mc)[:, 2 * hv + bb, 2:2 + H, 2:2 + W],
                vt4[:, bb], op=OP.mult)

    mm_c(evac_z, w_in8t)
    mm_c(evac_g, w_g8t)
    mm_c(evac_sg, w_a8t)

    # ---------------- E..I: half-batch pipelined middle section ----------
    taps5 = [(i, j) for i in range(5) for j in range(5)]
    a_f = gt_f  # per-hv slices of gt_f are re-used as a = rho*sigma

    # q broadcast (only needs LN1 sums; emit early)
    z32 = pool.tile([128, T], fp32, name="z32")
    nc.vector.memset(z32[:], 0.0)
    q_col = pool.tile([128, NCC, BL], fp32, name="q_col")
    for kc in range(NCC):
        nc.vector.tensor_scalar(
            q_col[:, kc, :], sview[:, 0, :, kc], 1.0 / float(HWN), None,
            op0=OP.mult)
        for b in range(BL):
            nc.vector.tensor_scalar(
                qt[:, kc, b, :], z32[:], q_col[:, kc, b:b + 1], None, op0=OP.add)

    # --- DW5^T(g) for both halves (keeps PE busy while DVE runs ladders) ---
    for hv in range(HV):
        for kc in range(NCC):
            ps = pp_mm.tile([128, 512], fp32, tag="mm", name=f"cgt{kc}{hv}")
            for ti, (i, j) in enumerate(taps5):
                fi = (4 - i) * 5 + (4 - j)          # flipped kernel index
                rhs = pad2(g_p, kc)[:, 2 * hv:2 * hv + 2, i:i + H, j:j + W]
                nc.tensor.matmul(
                    ps[:], dsp_t[:, kc, fi, :], rhs,
                    start=(ti == 0), stop=(ti == 24))
            nc.scalar.copy(gt_f[:, kc, hv * 512:(hv + 1) * 512], ps[:])
        # gbar = raw sum_hw g on ScalarE
        for kc in range(NCC):
            for b in range(2 * hv, 2 * hv + 2):
                gsc = scr.tile([128, HWN], bf16, tag="st_scr", name=f"gb{kc}{b}")
                nc.scalar.activation(
                    gsc[:].rearrange("p (h w) -> p h w", h=H),
                    int2(g_p, kc)[:, b], AF.Copy,
                    accum_out=gbar_c[:, kc, b:b + 1])

    def emit_seed_ladder(hv):
        hsl = slice(hv * 512, (hv + 1) * 512)
        bs = range(2 * hv, 2 * hv + 2)
        # P = z*Gt with fused s0 accumulation
        for kc in range(NCC):
            for b in bs:
                sl = slice(b * HWN, (b + 1) * HWN)
                nc.vector.scalar_tensor_tensor(
                    u_f[:, kc, sl], z_f[:, kc, sl], 1.0, gt_f[:, kc, sl],
                    op0=OP.bypass, op1=OP.mult, accum_out=s0_c[:, kc, b:b + 1])
        # a = rho*sigma into the (now dead) Gt slice
        for kc in range(NCC):
            nc.vector.tensor_scalar(a_f[:, kc, hsl], sg_f[:, kc, hsl], RHO,
                                    None, op0=OP.mult)
        # Q-ladder: kc 0/1 fused STT+accum on DVE, kc 2 via GpSimd TT +
        # ScalarE accum
        cur, nxt = u_f, q2
        for t in range(T):
            for kc in range(2):
                for b in bs:
                    sl = slice(b * HWN, (b + 1) * HWN)
                    nc.vector.scalar_tensor_tensor(
                        nxt[:, kc, sl], cur[:, kc, sl], RHO, sg_f[:, kc, sl],
                        op0=OP.mult, op1=OP.mult,
                        accum_out=st_all[:, kc, b, t:t + 1])
            nc.gpsimd.tensor_tensor(nxt[:, 2, hsl], cur[:, 2, hsl],
                                    a_f[:, 2, hsl], op=OP.mult)
            for b in bs:
                j_sc = scr.tile([128, HWN], bf16, tag="st_scr", name=f"st{t}{b}")
                nc.scalar.activation(
                    j_sc[:], nxt[:, 2, b * HWN:(b + 1) * HWN], AF.Copy,
                    accum_out=st_all[:, 2, b, t:t + 1])
            cur, nxt = nxt, cur

    def emit_gate(hv):
        bs = slice(2 * hv, 2 * hv + 2)
        wsl = slice(hv * 2 * T, (hv + 1) * 2 * T)
        inv = 1.0 / float(HWN)
        for kc in range(NCC):
            nc.vector.scalar_tensor_tensor(
                s0gb[:, kc, bs], gbar_c[:, kc, bs], b_sp_c[:, kc:kc + 1],
                s0_c[:, kc, bs], op0=OP.mult, op1=OP.add)
            nc.vector.tensor_scalar(
                s0gb[:, kc, bs], s0gb[:, kc, bs], inv, None, op0=OP.mult)
            for t in range(T):
                nc.vector.scalar_tensor_tensor(
                    kv[:, kc, bs, t], st_all[:, kc, bs, t], -inv,
                    s0gb[:, kc, bs], op0=OP.mult, op1=OP.add)
        # k through W_out (bf16 path)
        for mc in range(NCC):
            ps = pp_sm.tile([128, 2 * T], fp32, tag="sm", name=f"kwm{mc}{hv}")
            for kc in range(NCC):
                nc.tensor.matmul(
                    ps[:], w_out_t[:, kc, mc * 128:(mc + 1) * 128],
                    kv[:, kc, bs, :], start=(kc == 0), stop=(kc == NCC - 1))
            nc.vector.tensor_scalar(
                kw[:, mc, wsl], ps[:], b_out_c[:, mc:mc + 1], None, op0=OP.add)
        # gate hidden
        psg = pp_sm.tile([GH, 2 * T], fp32, tag="sm", name=f"psg{hv}")
        for i in range(2 * NCC):
            rhs = qt[:, i, bs, :] if i < NCC else kw[:, i - NCC, wsl]
            nc.tensor.matmul(psg[:], wg1_t[:, i, :], rhs,
                             start=(i == 0), stop=(i == 2 * NCC - 1))
        nc.scalar.activation(hg[:, wsl], psg[:], AF.Gelu_apprx_tanh,
                             bias=bg1_c[:])
        psl = pp_sm.tile([1, 2 * T], fp32, tag="sm", name=f"psl{hv}")
        nc.tensor.matmul(psl[:], wg2_t[:], hg[:, wsl], start=True, stop=True)
        nc.vector.scalar_tensor_tensor(
            logits[:, wsl], psl[:], bg2_c[:], prior_r[:, wsl],
            op0=OP.add, op1=OP.add)
        # softmax over t
        lv = logits[:, wsl].rearrange("p (b t) -> p b t", b=2)
        nc.vector.tensor_reduce(mx_r[:, bs], lv, axis=AX.X, op=OP.max)
        for b in range(2 * hv, 2 * hv + 2):
            nc.vector.tensor_scalar(
                esh[:, b * T:(b + 1) * T], logits[:, b * T:(b + 1) * T],
                mx_r[:, b:b + 1], None, op0=OP.subtract)
        nc.scalar.activation(esh[:, wsl], esh[:, wsl], AF.Exp)
        nc.vector.tensor_reduce(
            se_r[:, bs], esh[:, wsl].rearrange("p (b t) -> p b t", b=2),
            axis=AX.X, op=OP.add)
        nc.vector.reciprocal(se_r[:, bs], se_r[:, bs])
        for b in range(2 * hv, 2 * hv + 2):
            nc.vector.tensor_scalar(
                wneg[:, b * T:(b + 1) * T], esh[:, b * T:(b + 1) * T],
                se_r[:, b:b + 1], -XOS, op0=OP.mult, op1=OP.mult)
        nc.gpsimd.partition_broadcast(wbc[:, wsl], wneg[:, wsl], channels=128)

    def emit_horner(hv):
        # S <- (S + c_t)*a  from t=7..0 gives S = sum_t c_t a^{t+1}; with
        # c = -XOS*w (wbc) that is -XOS*W, and XOS*F = z*(XOS + S).
        acc = u_f  # ladder buffers are dead (T even -> cur == u_f)
        for kc in range(NCC):
            for b in range(2 * hv, 2 * hv + 2):
                sl = slice(b * HWN, (b + 1) * HWN)
                nc.vector.tensor_scalar(
                    acc[:, kc, sl], a_f[:, kc, sl],
                    wbc[:, b * T + 7:b * T + 8], None, op0=OP.mult)
                for t in range(6, -1, -1):
                    nc.vector.scalar_tensor_tensor(
                        acc[:, kc, sl], acc[:, kc, sl],
                        wbc[:, b * T + t:b * T + t + 1], a_f[:, kc, sl],
                        op0=OP.add, op1=OP.mult)
                nc.vector.scalar_tensor_tensor(
                    int2(f_p, kc)[:, b],
                    acc[:, kc, sl].rearrange("p (h w) -> p h w", h=H), XOS,
                    dense(z_f, kc)[:, b], op0=OP.add, op1=OP.mult)

    xo_rhs = xpos  # reuse xpos tile (fp8) as W_out rhs buffer; holds XOS*xo
    dw5f_ps = {}

    def emit_dw5f(hv):
        for kc in range(NCC):
            ps = pp_mm.tile([128, 512], fp32, tag="mm", name=f"cf{kc}{hv}")
            for ti, (i, j) in enumerate(taps5):
                rhs = pad2(f_p, kc)[:, 2 * hv:2 * hv + 2, i:i + H, j:j + W]
                nc.tensor.matmul(
                    ps[:], dsp_t[:, kc, ti, :], rhs,
                    start=(ti == 0), stop=(ti == 24))
            dw5f_ps[(kc, hv)] = ps

    def emit_xo_wout(hv):
        for kc in range(NCC):
            ps = dw5f_ps[(kc, hv)]
            ps4 = ps[:].rearrange("p (b h w) -> p b h w", b=2, h=H, w=W)
            for bb in range(2):
                b = 2 * hv + bb
                nc.vector.scalar_tensor_tensor(
                    dense(xo_rhs, kc)[:, b], ps4[:, bb], b_sp16_c[:, kc:kc + 1],
                    int2(g_p, kc)[:, b],
                    op0=OP.add, op1=OP.mult)
        for mc in range(NCC):
            ps = pp_mm.tile([128, 512], fp32, tag="mm", name=f"wo{mc}{hv}")
            nc.tensor.matmul(
                ps[:], w_out8t[:, 0:2, mc * 128:(mc + 1) * 128],
                xo_rhs[:, 0:2, hv * 512:(hv + 1) * 512],
                start=True, stop=False, perf_mode=DR)
            nc.tensor.matmul(
                ps[:], w_out8t[:, 2, mc * 128:(mc + 1) * 128],
                xo_rhs[:, 2, hv * 512:(hv + 1) * 512],
                start=False, stop=True)
            # psum holds WS*(x_out - b_out); rescale on ScalarE then add x
            xot = scr.tile([128, 512], bf16, tag="xot", name=f"xot{mc}{hv}")
            nc.scalar.activation(xot[:], ps[:], AF.Identity,
                                 bias=b_out_c[:, mc:mc + 1], scale=SCL)
            nc.vector.tensor_tensor(
                out1[:, mc, hv * 512:(hv + 1) * 512], xot[:],
                x_cm[:, mc, hv * 512:(hv + 1) * 512], op=OP.add)

    # pipeline: all DVE field work (ladder/gate/Horner per half) is emitted
    # before the PSUM evacuations so the in-order DVE queue never parks
    # behind an evac that waits on PE convolutions.
    emit_seed_ladder(0)
    emit_gate(0)
    emit_horner(0)
    emit_seed_ladder(1)
    emit_gate(1)
    emit_horner(1)
    emit_dw5f(0)
    emit_dw5f(1)
    emit_xo_wout(0)
    emit_xo_wout(1)

    # ---------------- J: LN2 ----------------
    o1b = q2  # bf16 copy of out1 (ladder buffer is dead)
    for kc in range(NCC):
        nc.scalar.copy(o1b[:, kc, :], out1[:, kc, :])
        nc.vector.tensor_tensor(u_f[:, kc, :], o1b[:, kc, :], o1b[:, kc, :],
                                op=OP.mult)   # squares into u_f
    for hv in range(HV):
        ps0 = pp_sm.tile([1, 512], fp32, tag="sm", name=f"l2s{hv}")
        for kc in range(NCC):
            nc.tensor.matmul(ps0[:], ones_c[:], o1b[:, kc, hv * 512:(hv + 1) * 512],
                             start=(kc == 0), stop=(kc == NCC - 1))
        nc.scalar.copy(stats2[:, 0, hv * 512:(hv + 1) * 512], ps0[:])
        ps1 = pp_sm.tile([1, 512], fp32, tag="sm", name=f"l2q{hv}")
        for kc in range(NCC):
            nc.tensor.matmul(ps1[:], ones_c[:], u_f[:, kc, hv * 512:(hv + 1) * 512],
                             start=(kc == 0), stop=(kc == NCC - 1))
        nc.scalar.copy(stats2[:, 1, hv * 512:(hv + 1) * 512], ps1[:])
    nc.scalar.mul(stats2[:, 0, :], stats2[:, 0, :], 1.0 / float(C))   # mu
    nc.scalar.mul(stats2[:, 1, :], stats2[:, 1, :], 1.0 / float(C))   # E[x^2]
    nc.vector.tensor_tensor(work2[:], stats2[:, 0, :], stats2[:, 0, :], op=OP.mult)
    nc.vector.tensor_tensor(work2[:], stats2[:, 1, :], work2[:], op=OP.subtract)
    nc.vector.tensor_scalar(work2[:], work2[:], EPS, None, op0=OP.add)
    # rstd = exp(-0.5*ln(var)) on ScalarE (avoids slow 1-partition DVE recip)
    nc.scalar.activation(lnv2[:], work2[:], AF.Ln)
    nc.scalar.activation(work2[:], lnv2[:], AF.Exp, scale=-0.5)
    nc.vector.tensor_copy(rhsS[:], work2[:])
    nc.vector.tensor_tensor(stats2[:, 0, :], stats2[:, 0, :], work2[:], op=OP.mult)
    nc.vector.tensor_scalar(stats2[:, 0, :], stats2[:, 0, :], -1.0, None,
                            op0=OP.mult)
    nc.vector.tensor_copy(rhsM[:], stats2[:, 0, :])
    for kc in range(NCC):
        for hv in range(HV):
            psS = pp_mm.tile([128, 512], fp32, tag="mm", name=f"lnS{kc}{hv}")
            nc.tensor.matmul(psS[:], g2_t[0:1, kc, :],
                             rhsS[:, hv * 512:(hv + 1) * 512],
                             start=True, stop=True)
            psB = pp_mm.tile([128, 512], fp32, tag="mm", name=f"lnB{kc}{hv}")
            nc.tensor.matmul(psB[:], g2_t[0:1, kc, :],
                             rhsM[:, hv * 512:(hv + 1) * 512],
                             start=True, stop=True)
            nc.vector.tensor_tensor(
                z_f[:, kc, hv * 512:(hv + 1) * 512],
                o1b[:, kc, hv * 512:(hv + 1) * 512], psS[:], op=OP.mult)
            nc.vector.scalar_tensor_tensor(
                yn8[:, kc, hv * 512:(hv + 1) * 512],
                z_f[:, kc, hv * 512:(hv + 1) * 512], be2_c[:, kc:kc + 1],
                psB[:], op0=OP.add, op1=OP.add)

    # ---------------- K: MLP (fp8 DoubleRow) ----------------
    for jc in range(NHC):
        for hv in range(HV):
            ps = pp_mm.tile([128, 512], fp32, tag="mm", name=f"w1_{jc}{hv}")
            nc.tensor.matmul(
                ps[:], w1_8t[:, 0:2, jc * 128:(jc + 1) * 128],
                yn8[:, 0:2, hv * 512:(hv + 1) * 512],
                start=True, stop=False, perf_mode=DR)
            nc.tensor.matmul(
                ps[:], w1_8t[:, 2, jc * 128:(jc + 1) * 128],
                yn8[:, 2, hv * 512:(hv + 1) * 512],
                start=False, stop=True)
            ps4 = ps[:].rearrange("p (b h w) -> p b h w", b=2, h=H, w=W)
            for bb in range(2):
                nc.scalar.activation(
                    pad1(h1p, jc)[:, 2 * hv + bb, 1:1 + H, 1:1 + W], ps4[:, bb],
                    AF.Identity, bias=b1_c[:, jc:jc + 1], scale=SCL)
    taps3 = [(i, j) for i in range(3) for j in range(3)]
    for jc in range(NHC):
        ddw_t = dpool.tile([128, 9, 128], f8, tag="ddw", name=f"ddw{jc}")
        nc.sync.dma_start(ddw_t[:], d["ddw"][:, jc])
        for hv in range(HV):
            ps = pp_mm.tile([128, 512], fp32, tag="mm", name=f"cdw{jc}{hv}")
            for ti, (i, j) in enumerate(taps3):
                rhs = pad1(h1p, jc)[:, 2 * hv:2 * hv + 2, i:i + H, j:j + W]
                nc.tensor.matmul(ps[:], ddw_t[:, ti, :], rhs,
                                 start=(ti == 0), stop=(ti == 8))
            nc.scalar.activation(
                h1g[:, jc, hv * 512:(hv + 1) * 512], ps[:],
                AF.Gelu_apprx_tanh, bias=bdw_c[:, jc:jc + 1], scale=SCL)
    for mc in range(NCC):
        for hv in range(HV):
            ps = pp_mm.tile([128, 512], fp32, tag="mm", name=f"w2_{mc}{hv}")
            for jp in range(NHC // 2):
                nc.tensor.matmul(
                    ps[:],
                    w2_8t[:, 2 * jp:2 * jp + 2, mc * 128:(mc + 1) * 128],
                    h1g[:, 2 * jp:2 * jp + 2, hv * 512:(hv + 1) * 512],
                    start=(jp == 0), stop=(jp == NHC // 2 - 1), perf_mode=DR)
            mot = scr.tile([128, 512], bf16, tag="xot", name=f"mot{mc}{hv}")
            nc.scalar.activation(mot[:], ps[:], AF.Identity,
                                 bias=b2_c[:, mc:mc + 1], scale=SCL)
            nc.vector.tensor_tensor(
                out1[:, mc, hv * 512:(hv + 1) * 512],
                out1[:, mc, hv * 512:(hv + 1) * 512], mot[:], op=OP.add)
        nc.sync.dma_start(out_d[:, mc, :], out1[:, mc, :])

    ctx.close()


# ------------------------------------------------------------------
# host side
# ------------------------------------------------------------------

def _diagify(k2d, nchunks, add_identity=False):
    """k2d: (KH, KW, 1, Cn) -> (KH*KW, nchunks, 128, 128) bf16 diagonals."""
    kh, kw = k2d.shape[0], k2d.shape[1]
    out = np.zeros((kh * kw, nchunks, 128, 128), dtype=BF16)
    idx = np.arange(128)
    for t in range(kh * kw):
        vals = k2d[t // kw, t % kw, 0].astype(np.float32)
        if add_identity and t == (kh * kw) // 2:
            vals = vals + 1.0
        for c in range(nchunks):
            out[t, c, idx, idx] = vals[c * 128:(c + 1) * 128].astype(BF16)
    return out


def _f8(a):
    return np.clip(np.asarray(a, np.float32) * WS, -240.0, 240.0).astype(FP8)


def _prep_shared(w):
    """Build the shared (weight) input map from the raw input dict."""
    f32 = np.float32
    m = {}
    def pm(a):  # [k,128,...] -> [128,k,...] contiguous
        return np.ascontiguousarray(np.moveaxis(a, 1, 0))

    m["w_in8"] = _f8(pm(w["W_in"].astype(f32).reshape(NCC, 128, C)))
    m["w_a8"] = _f8(pm(w["W_a"].astype(f32).reshape(NCC, 128, C)))
    m["w_g8"] = _f8(pm(w["W_g"].astype(f32).reshape(NCC, 128, C)))
    wo = pm(w["W_out"].astype(f32).reshape(NCC, 128, C))
    m["w_out"] = wo.astype(BF16)
    # W_out fp8 carries WS/XOS so psum = WS * (W_out @ xo)
    m["w_out8"] = np.clip(wo * (WS / XOS), -240.0, 240.0).astype(FP8)
    m["w1_8"] = _f8(pm(w["W1"].astype(f32).reshape(NCC, 128, HID)))
    m["w2_8"] = _f8(pm(w["W2"].astype(f32).reshape(NHC, 128, C)))
    m["wg1"] = pm(w["Wg1"].astype(f32).reshape(2 * NCC, 128, GH)).astype(BF16)
    m["wg2"] = w["Wg2"].astype(f32).reshape(GH, 1).astype(BF16)
    m["dpos"] = np.ascontiguousarray(
        _diagify(np.asarray(w["w_pos"]), NCC, add_identity=True)
        .transpose(2, 1, 0, 3))
    m["dsp"] = np.ascontiguousarray(
        _diagify(np.asarray(w["k_sp"]), NCC).transpose(2, 1, 0, 3))
    # dw3 diagonals for the HID conv: [128, NHC, 9, 128] fp8 (x WS)
    vals9 = np.asarray(w["wdw"], f32).reshape(9, NHC, 128)
    ddw = np.zeros((128, NHC, 9, 128), dtype=FP8)
    idx = np.arange(128)
    ddw[idx, :, :, idx] = np.clip(
        vals9.transpose(2, 1, 0) * WS, -240.0, 240.0).astype(FP8)
    m["ddw"] = np.ascontiguousarray(ddw)
    # sp1 = 1 + conv3(ones) per channel: [128, NCC, HWN]
    wp = np.asarray(w["w_pos"], f32)[:, :, 0, :]           # (3,3,C)
    ones_im = np.ones((H, W), f32)
    s_acc = np.zeros((C, H, W), f32)
    for i in range(3):
        for j in range(3):
            shifted = np.zeros((H, W), f32)
            ys = slice(max(0, 1 - i), min(H, H + 1 - i))
            xs = slice(max(0, 1 - j), min(W, W + 1 - j))
            ys_s = slice(max(0, i - 1), min(H, H + i - 1))
            xs_s = slice(max(0, j - 1), min(W, W + j - 1))
            shifted[ys, xs] = ones_im[ys_s, xs_s]
            s_acc += wp[i, j][:, None, None] * shifted[None]
    sp1 = 1.0 + s_acc                                       # (C, H, W)
    m["sp1"] = np.ascontiguousarray(
        sp1.reshape(NCC, 128, HWN).transpose(1, 0, 2)).astype(BF16)
    for src, dst, n in [("b_in", "b_in", NCC), ("b_a", "b_a", NCC),
                        ("b_g", "b_g", NCC), ("b_sp", "b_sp", NCC),
                        ("b_out", "b_out", NCC), ("b2", "b2", NCC),
                        ("gamma1", "gamma1", NCC), ("beta1", "beta1", NCC),
                        ("b1", "b1", NHC), ("bdw", "bdw", NHC)]:
        m[dst] = np.ascontiguousarray(np.asarray(w[src], f32).reshape(n, 128).T)
    m["b_sp16"] = np.ascontiguousarray(
        (np.asarray(w["b_sp"], f32) * XOS).reshape(NCC, 128).T)
    m["b_pos"] = np.ascontiguousarray(
        np.asarray(w["b_pos"], f32).reshape(NCC, 128).T)
    m["g2r"] = np.asarray(w["gamma2"], f32).reshape(1, NCC, 128).astype(BF16)
    m["be2"] = np.ascontiguousarray(
        np.asarray(w["beta2"], f32).reshape(NCC, 128).T)
    m["bg1"] = np.asarray(w["bg1"], f32).reshape(GH, 1)
    m["bg2"] = np.asarray(w["bg2"], f32).reshape(1, 1)
    prior = np.zeros((T,), f32)
    prior[-1] = 4.0
    m["prior"] = np.tile(prior, BL)[None, :]
    return m


TRACE = False       # set True (e.g. from test.py) to capture an NTFF profile
LAST_RES = None


def kernel(**inputs):
    global _PROG, LAST_RES
    from concourse.bass_utils import run_bass_kernel_spmd

    if _PROG is None:
        _PROG = _build_program()
    nc = _PROG

    shared = _prep_shared(inputs)
    x = np.asarray(inputs["x"], np.float32)
    in_maps = []
    for i in range(NCORES):
        im = dict(shared)
        xs = x[i * BL:(i + 1) * BL].reshape(NTOK, C)
        # channel-major bf16 spine [128, NCC, NTOK]
        xcm = xs.T.reshape(NCC, 128, NTOK).transpose(1, 0, 2)
        im["x_cm"] = np.ascontiguousarray(xcm).astype(BF16)
        # zero-padded bf16 copy for the 3x3 positional conv
        xb = xs.astype(BF16).reshape(BL, H, W, C)
        xp = np.zeros((BL, H1, W1P, C), BF16)
        xp[:, 1:1 + H, 1:1 + W, :] = xb
        # -> [128, NCC, BL*H1*W1P]
        xp = xp.reshape(BL * H1 * W1P, C).T.reshape(NCC, 128, F1)
        im["x_pad"] = np.ascontiguousarray(xp.transpose(1, 0, 2))
        in_maps.append(im)

    res = run_bass_kernel_spmd(nc, in_maps, core_ids=list(range(NCORES)),
                               trace=TRACE)
    LAST_RES = res
    outs = []
    for r in res.results:
        o = r["out"]                      # [128, NCC, NTOK]
        o = o.transpose(1, 0, 2).reshape(C, NTOK).T    # [NTOK, C]
        outs.append(o.reshape(BL, H, W, C))
    return np.concatenate(outs, axis=0)


# revision 11
# speedup vs baseline: 1.0017x; 1.0017x over previous
"""Trainium2 Bass kernel for nn_CSSMSHViT_60043642798201.

Strategy
--------
The reference repeats the input image over a time axis T=8 and runs a gated
scalar recurrence over T.  Because the input is constant over T the whole
temporal structure collapses algebraically:

    h_t = (1 - a^{t+1}) z          (closed form of the scan)

so the per-timestep fields are never materialised.  The kernel computes

  LN1 (global per-batch) -> +3x3 depthwise pos conv -> z/sigma/g projections
  -> power ladder u_t = a^t z with fused per-batch reductions St = sum(u_t*Gt)
     where Gt = DW5^T(g)  (adjoint trick: mean(DW5(u)*g) = mean(u*DW5^T(g)))
  -> tiny gate MLP -> softmax weights w
  -> F = z - sum_t w_t u_{t+1};  x_out = (DW5(F)+b_sp)*g @ W_out + b_out
  -> out1 = x + x_out -> LN2 -> MLP with 3x3 depthwise conv -> out.

Sharding: pure data-parallel over batch (32 = 8 cores x 4), no collectives.

Layouts: channel-major [C_partition, (b, h, w)] SBUF fields (the host does
the NHWC <-> channel-major transposes); depthwise convs run on the
TensorEngine as diagonal-weight matmuls accumulated over taps in PSUM with
shifted access patterns into zero-padded buffers.  Dense projections run in
fp8 (weights pre-scaled x256 on the host, rescaled in the PSUM evacuation)
using DoubleRow perf mode to contract two 128-channel chunks per pass.  The
LN1 affine is commuted through the (linear) positional conv so the conv runs
on raw x and needn't wait for LN1 statistics.  The middle section is
pipelined over two batch-halves, the power ladder fuses its per-batch
reductions into scalar_tensor_tensor accum outputs, and the third channel
chunk of the ladder/Horner recurrences runs on the (otherwise idle) GpSimd
engine.
"""

import numpy as np
import ml_dtypes

BF16 = ml_dtypes.bfloat16
FP8 = ml_dtypes.float8_e4m3

# problem constants
B, T, H, W, C = 32, 8, 16, 16, 384
KS = 5
HID = 4 * C
GH = max(C // 4, 8)
RHO = 0.999
EPS = 1e-6

NCORES = 8
BL = B // NCORES            # batches per core = 4
HWN = H * W                 # 256 tokens per image
NTOK = BL * HWN             # 1024 tokens per core
NCC = C // 128              # 3 channel chunks
NHC = HID // 128            # 12 hidden chunks

WS = 256.0                  # fp8 weight pre-scale (avoids subnormals)
XOS = 16.0                  # F/x_out field pre-scale for fp8 range

# padded geometries (channel-major fields, free layout (b, hp, wp))
H1, W1P = 18, 18            # pad-1 buffers (3x3 convs)
F1 = BL * H1 * W1P
H2, W2P = 20, 20            # pad-2 buffers (5x5 convs)
F2 = BL * H2 * W2P

_PROG = None  # cached compiled program


def _build_program():
    import concourse.bass as bass
    import concourse.tile as tile
    from concourse import bacc, mybir

    fp32 = mybir.dt.float32
    bf16 = mybir.dt.bfloat16
    f8 = mybir.dt.float8e4
    AF = mybir.ActivationFunctionType
    OP = mybir.AluOpType
    AX = mybir.AxisListType
    DR = mybir.MatmulPerfMode.DoubleRow

    nc = bacc.Bacc("TRN2", target_bir_lowering=False)

    # ---------------- DRAM tensors ----------------
    d = {}
    d["x_cm"] = nc.dram_tensor("x_cm", [128, NCC, NTOK], bf16, kind="ExternalInput")
    d["x_pad"] = nc.dram_tensor("x_pad", [128, NCC, F1], bf16, kind="ExternalInput")
    d["sp1"] = nc.dram_tensor("sp1", [128, NCC, HWN], bf16, kind="ExternalInput")
    # fp8 matmul weights (pre-scaled x256), chunked [128, kchunks, M]
    d["w_in8"] = nc.dram_tensor("w_in8", [128, NCC, C], f8, kind="ExternalInput")
    d["w_a8"] = nc.dram_tensor("w_a8", [128, NCC, C], f8, kind="ExternalInput")
    d["w_g8"] = nc.dram_tensor("w_g8", [128, NCC, C], f8, kind="ExternalInput")
    d["w_out8"] = nc.dram_tensor("w_out8", [128, NCC, C], f8, kind="ExternalInput")
    d["w1_8"] = nc.dram_tensor("w1_8", [128, NCC, HID], f8, kind="ExternalInput")
    d["w2_8"] = nc.dram_tensor("w2_8", [128, NHC, C], f8, kind="ExternalInput")
    d["w_out"] = nc.dram_tensor("w_out", [128, NCC, C], bf16, kind="ExternalInput")
    d["wg1"] = nc.dram_tensor("wg1", [128, 2 * NCC, GH], bf16, kind="ExternalInput")
    d["wg2"] = nc.dram_tensor("wg2", [GH, 1], bf16, kind="ExternalInput")
    # diagonalised depthwise kernels (partition-major, contiguous per partition)
    d["dpos"] = nc.dram_tensor("dpos", [128, NCC, 9, 128], bf16, kind="ExternalInput")
    d["dsp"] = nc.dram_tensor("dsp", [128, NCC, 25, 128], bf16, kind="ExternalInput")
    d["ddw"] = nc.dram_tensor("ddw", [128, NHC, 9, 128], f8, kind="ExternalInput")
    # per-channel vectors [128, nchunks] fp32
    for nm in ["b_in", "b_a", "b_g", "b_sp", "b_sp16", "b_out", "b2",
               "gamma1", "beta1", "b_pos"]:
        d[nm] = nc.dram_tensor(nm, [128, NCC], fp32, kind="ExternalInput")
    d["b1"] = nc.dram_tensor("b1", [128, NHC], fp32, kind="ExternalInput")
    d["bdw"] = nc.dram_tensor("bdw", [128, NHC], fp32, kind="ExternalInput")
    d["g2c"] = nc.dram_tensor("g2c", [128, NCC], fp32, kind="ExternalInput")
    d["be2"] = nc.dram_tensor("be2", [128, NCC], fp32, kind="ExternalInput")
    d["bg1"] = nc.dram_tensor("bg1", [GH, 1], fp32, kind="ExternalInput")
    d["bg2"] = nc.dram_tensor("bg2", [1, 1], fp32, kind="ExternalInput")
    d["prior"] = nc.dram_tensor("prior", [1, BL * T], fp32, kind="ExternalInput")
    out_d = nc.dram_tensor("out", [128, NCC, NTOK], fp32, kind="ExternalOutput")

    with tile.TileContext(nc) as tc:
        _emit(nc, tc, d, out_d, mybir, bass, fp32, bf16, f8, AF, OP, AX, DR)

    nc.compile()
    return nc


def _emit(nc, tc, d, out_d, mybir, bass, fp32, bf16, f8, AF, OP, AX, DR):
    from contextlib import ExitStack
    ctx = ExitStack()

    pool = ctx.enter_context(tc.tile_pool(name="persist", bufs=1))
    dpool = ctx.enter_context(tc.tile_pool(name="ddwst", bufs=3))
    scr = ctx.enter_context(tc.tile_pool(name="scratch", bufs=2))
    pp_mm = ctx.enter_context(tc.tile_pool(name="pp_mm", bufs=6, space="PSUM"))
    pp_sm = ctx.enter_context(tc.tile_pool(name="pp_sm", bufs=1, space="PSUM"))

    import concourse.bass_isa as bass_isa
    RADD = bass_isa.ReduceOp.add
    SCL = 1.0 / WS

    # ---------------- persistent field tiles ----------------
    x_cm = pool.tile([128, NCC, NTOK], bf16, name="x_cm")
    xn0p = pool.tile([128, NCC, F1], bf16, name="xn0p")            # padded raw x
    sp1 = pool.tile([128, NCC, HWN], bf16, name="sp1")             # 1 + conv3(1)
    xpos = pool.tile([128, NCC, NTOK], f8, name="xpos")            # also xo_rhs
    z_f = pool.tile([128, NCC, NTOK], bf16, name="z_f")            # reused LN2 tmp
    sg_f = pool.tile([128, NCC, NTOK], bf16, name="sg_f")
    g_p = pool.tile([128, NCC, F2], bf16, name="g_p")              # padded silu gate
    gt_f = pool.tile([128, NCC, NTOK], bf16, name="gt_f")          # Gt; later a=rho*sig
    u_f = pool.tile([128, NCC, NTOK], bf16, name="u_f")            # ladder A/acc/sq
    q2 = pool.tile([128, NCC, NTOK], bf16, name="q2")              # ladder B / o1b
    f_p = pool.tile([128, NCC, F2], bf16, name="f_p")              # padded XOS*F field
    out1 = pool.tile([128, NCC, NTOK], fp32, name="out1")          # also final out
    yn8 = pool.tile([128, NCC, NTOK], f8, name="yn8")
    h1p = pool.tile([128, NHC, F1], f8, name="h1p")                # padded MLP hidden
    h1g = pool.tile([128, NHC, NTOK], f8, name="h1g")              # dense gelu(conv)

    # weights
    w_in8t = pool.tile([128, NCC, C], f8, name="w_in8t")
    w_a8t = pool.tile([128, NCC, C], f8, name="w_a8t")
    w_g8t = pool.tile([128, NCC, C], f8, name="w_g8t")
    w_out8t = pool.tile([128, NCC, C], f8, name="w_out8t")
    w1_8t = pool.tile([128, NCC, HID], f8, name="w1_8t")
    w2_8t = pool.tile([128, NHC, C], f8, name="w2_8t")
    w_out_t = pool.tile([128, NCC, C], bf16, name="w_out_t")       # for gate k path
    wg1_t = pool.tile([128, 2 * NCC, GH], bf16, name="wg1_t")
    wg2_t = pool.tile([GH, 1], bf16, name="wg2_t")
    dsp_t = pool.tile([128, NCC, 25, 128], bf16, name="dsp_t")     # resident 5x5 diags
    dpos_t = pool.tile([128, NCC, 9, 128], bf16, name="dpos_t")    # resident 3x3 diags

    # vectors
    b_in_c = pool.tile([128, NCC], fp32, name="b_in_c")
    b_a_c = pool.tile([128, NCC], fp32, name="b_a_c")
    b_g_c = pool.tile([128, NCC], fp32, name="b_g_c")
    b_sp_c = pool.tile([128, NCC], fp32, name="b_sp_c")
    b_sp16_c = pool.tile([128, NCC], fp32, name="b_sp16_c")
    b_pos_c = pool.tile([128, NCC], fp32, name="b_pos_c")
    b_out_c = pool.tile([128, NCC], fp32, name="b_out_c")
    b2_c = pool.tile([128, NCC], fp32, name="b2_c")
    g1_c = pool.tile([128, NCC], fp32, name="g1_c")
    be1_c = pool.tile([128, NCC], fp32, name="be1_c")
    b1_c = pool.tile([128, NHC], fp32, name="b1_c")
    bdw_c = pool.tile([128, NHC], fp32, name="bdw_c")
    g2c = pool.tile([128, NCC], fp32, name="g2c")
    be2_c = pool.tile([128, NCC], fp32, name="be2_c")
    bg1_c = pool.tile([GH, 1], fp32, name="bg1_c")
    bg2_c = pool.tile([1, 1], fp32, name="bg2_c")
    prior_r = pool.tile([1, BL * T], fp32, name="prior_r")

    # small working tiles
    ones_c = pool.tile([128, 1], bf16, name="ones_c")
    sums = pool.tile([128, 24], fp32, name="sums")       # stat*12 + b*3 + kc
    ar = pool.tile([128, 24], fp32, name="ar")
    tot = pool.tile([128, 2, BL], fp32, name="tot")
    m_col = pool.tile([128, BL], fp32, name="m_col")
    e2_col = pool.tile([128, BL], fp32, name="e2_col")
    var_col = pool.tile([128, BL], fp32, name="var_col")
    rstd_col = pool.tile([128, BL], fp32, name="rstd_col")
    sc_col = pool.tile([128, NCC, BL], fp32, name="sc_col")
    bi_col = pool.tile([128, NCC, BL], fp32, name="bi_col")
    tmp_col = pool.tile([128, BL], fp32, name="tmp_col")
    st_all = pool.tile([128, NCC, BL, T], fp32, name="st_all")
    s0_c = pool.tile([128, NCC, BL], fp32, name="s0_c")
    gbar_c = pool.tile([128, NCC, BL], fp32, name="gbar_c")
    s0gb = pool.tile([128, NCC, BL], fp32, name="s0gb")
    kv = pool.tile([128, NCC, BL, T], bf16, name="kv")
    qt = pool.tile([128, NCC, BL, T], bf16, name="qt")
    kw = pool.tile([128, NCC, BL * T], bf16, name="kw")
    hg = pool.tile([GH, BL * T], bf16, name="hg")
    logits = pool.tile([1, BL * T], fp32, name="logits")
    mx_r = pool.tile([1, BL], fp32, name="mx_r")
    esh = pool.tile([1, BL * T], fp32, name="esh")
    se_r = pool.tile([1, BL], fp32, name="se_r")
    wneg = pool.tile([1, BL * T], fp32, name="wneg")
    wbc = pool.tile([128, BL * T], fp32, name="wbc")
    stats2 = pool.tile([1, 2, NTOK], fp32, name="stats2")   # LN2 sums
    work2 = pool.tile([1, NTOK], fp32, name="work2")
    lnv2 = pool.tile([1, NTOK], fp32, name="lnv2")
    rhsS = pool.tile([1, NTOK], bf16, name="rhsS")          # rstd
    rhsM = pool.tile([1, NTOK], bf16, name="rhsM")          # -mu*rstd
    rstdB = pool.tile([128, NTOK], bf16, name="rstdB")
    mrstdB = pool.tile([128, NTOK], bf16, name="mrstdB")

    # ---------------- loads (conv operands first; then consumption order) ----
    for kc in range(NCC):
        nc.sync.dma_start(xn0p[:, kc, :], d["x_pad"][:, kc, :])
    for kc in range(NCC):
        nc.sync.dma_start(dpos_t[:, kc], d["dpos"][:, kc])
    for kc in range(NCC):
        nc.sync.dma_start(x_cm[:, kc, :], d["x_cm"][:, kc, :])
    nc.sync.dma_start(sp1[:], d["sp1"][:])

    def ld(tile_ap, dram):
        nc.sync.dma_start(tile_ap[:], dram[:])

    for nm, t_ in [("gamma1", g1_c), ("beta1", be1_c), ("b_pos", b_pos_c),
                   ("b_in", b_in_c), ("b_a", b_a_c), ("b_g", b_g_c),
                   ("b_sp", b_sp_c), ("b_sp16", b_sp16_c),
                   ("b_out", b_out_c), ("b2", b2_c)]:
        ld(t_, d[nm])
    ld(w_g8t, d["w_g8"])
    ld(w_in8t, d["w_in8"])
    ld(w_a8t, d["w_a8"])
    for kc in range(NCC):
        nc.sync.dma_start(dsp_t[:, kc], d["dsp"][:, kc])
    ld(w_out_t, d["w_out"])
    ld(w_out8t, d["w_out8"])
    ld(wg1_t, d["wg1"])
    nc.sync.dma_start(wg2_t[:], d["wg2"][:])
    ld(g2c, d["g2c"])
    ld(be2_c, d["be2"])
    nc.sync.dma_start(bg1_c[:], d["bg1"][:])
    nc.sync.dma_start(bg2_c[:], d["bg2"][:])
    nc.sync.dma_start(prior_r[:], d["prior"][:])
    ld(w1_8t, d["w1_8"])
    ld(w2_8t, d["w2_8"])
    ld(b1_c, d["b1"])
    ld(bdw_c, d["bdw"])

    nc.vector.memset(ones_c[:], 1.0)

    # zero padded buffers (borders must stay zero)
    nc.gpsimd.memset(g_p[:].rearrange("p a b -> p (a b)"), 0.0)
    nc.gpsimd.memset(f_p[:].rearrange("p a b -> p (a b)"), 0.0)
    nc.gpsimd.memset(h1p[:].rearrange("p a b -> p (a b)"), 0.0)

    # view helpers -------------------------------------------------
    def pad1(tile_, j):           # -> [128, BL, H1, W1P] for chunk j
        return tile_[:, j, :].rearrange("p (b h w) -> p b h w", b=BL, h=H1, w=W1P)

    def pad2(tile_, j):
        return tile_[:, j, :].rearrange("p (b h w) -> p b h w", b=BL, h=H2, w=W2P)

    def dense(tile_, j):          # -> [128, BL, H, W]
        return tile_[:, j, :].rearrange("p (b h w) -> p b h w", b=BL, h=H, w=W)

    def int1(tile_, j):           # pad1 interior
        return pad1(tile_, j)[:, :, 1:1 + H, 1:1 + W]

    def int2(tile_, j):
        return pad2(tile_, j)[:, :, 2:2 + H, 2:2 + W]

    HV = NTOK // 512              # 2 halves (2 batches each)

    # ---------------- B: LN1 stats ----------------
    sview = sums[:].rearrange("p (s b k) -> p s b k", s=2, b=BL, k=NCC)
    for kc in range(NCC):
        nc.vector.tensor_reduce(
            sview[:, 0, :, kc],
            x_cm[:, kc, :].rearrange("p (b n) -> p b n", b=BL),
            axis=AX.X, op=OP.add)
        for b in range(BL):
            s_sc = scr.tile([128, HWN], bf16, tag="st_scr", name=f"sxx{kc}{b}")
            nc.scalar.activation(
                s_sc[:], x_cm[:, kc, b * HWN:(b + 1) * HWN], AF.Square,
                accum_out=sview[:, 1, b, kc:kc + 1])
    nc.gpsimd.partition_all_reduce(ar[:], sums[:], channels=128, reduce_op=RADD)
    nc.vector.tensor_reduce(
        tot[:], ar[:].rearrange("p (s b k) -> p s b k", s=2, b=BL, k=NCC),
        axis=AX.X, op=OP.add)
    NB = float(HWN * C)
    nc.vector.tensor_scalar(m_col[:], tot[:, 0, :], 1.0 / NB, None, op0=OP.mult)
    nc.vector.tensor_scalar(e2_col[:], tot[:, 1, :], 1.0 / NB, None, op0=OP.mult)
    nc.vector.tensor_tensor(tmp_col[:], m_col[:], m_col[:], op=OP.mult)
    nc.vector.tensor_tensor(var_col[:], e2_col[:], tmp_col[:], op=OP.subtract)
    nc.vector.tensor_scalar(var_col[:], var_col[:], EPS, None, op0=OP.add)
    nc.scalar.sqrt(var_col[:], var_col[:])
    nc.vector.reciprocal(rstd_col[:], var_col[:])
    for kc in range(NCC):
        nc.vector.tensor_scalar(
            sc_col[:, kc, :], rstd_col[:], g1_c[:, kc:kc + 1], None, op0=OP.mult)
        nc.vector.tensor_tensor(tmp_col[:], m_col[:], sc_col[:, kc, :], op=OP.mult)
        nc.vector.tensor_scalar(
            bi_col[:, kc, :], tmp_col[:], be1_c[:, kc:kc + 1], -1.0,
            op0=OP.subtract, op1=OP.mult)

    # ---------------- C: positional conv on raw x (identity-augmented) -------
    # xpos = sc*(x + conv3(x)) + bi*(1 + conv3(1)) + b_pos   (LN1 affine
    # commuted through the linear conv; dpos has +1 on the centre tap).
    for kc in range(NCC):
        for hv in range(HV):
            ps = pp_mm.tile([128, 512], fp32, tag="mm", name=f"cpos{kc}{hv}")
            for ti, (i, j) in enumerate([(a, b) for a in range(3) for b in range(3)]):
                rhs = pad1(xn0p, kc)[:, 2 * hv:2 * hv + 2, i:i + H, j:j + W]
                nc.tensor.matmul(
                    ps[:], dpos_t[:, kc, ti, :], rhs,
                    start=(ti == 0), stop=(ti == 8))
            ps4 = ps[:].rearrange("p (b h w) -> p b h w", b=2, h=H, w=W)
            for bb in range(2):
                b = 2 * hv + bb
                bia = scr.tile([128, HWN], bf16, tag="bia", name=f"bia{kc}{b}")
                nc.vector.tensor_scalar(
                    bia[:], sp1[:, kc, :], bi_col[:, kc, b:b + 1],
                    b_pos_c[:, kc:kc + 1], op0=OP.mult, op1=OP.add)
                nc.vector.scalar_tensor_tensor(
                    dense(xpos, kc)[:, b], ps4[:, bb], sc_col[:, kc, b:b + 1],
                    bia[:].rearrange("p (h w) -> p h w", h=H),
                    op0=OP.mult, op1=OP.add)

    # ---------------- D: z / sigma / g projections (fp8 DoubleRow) ----------
    def mm_c(dst_evac, w8t):
        for mc in range(NCC):
            for hv in range(HV):
                ps = pp_mm.tile([128, 512], fp32, tag="mm",
                                name=f"mmc_{id(w8t)}_{mc}_{hv}")
                nc.tensor.matmul(
                    ps[:], w8t[:, 0:2, mc * 128:(mc + 1) * 128],
                    xpos[:, 0:2, hv * 512:(hv + 1) * 512],
                    start=True, stop=False, perf_mode=DR)
                nc.tensor.matmul(
                    ps[:], w8t[:, 2, mc * 128:(mc + 1) * 128],
                    xpos[:, 2, hv * 512:(hv + 1) * 512],
                    start=False, stop=True)
                dst_evac(mc, hv, ps)

    def evac_z(mc, hv, ps):
        nc.scalar.activation(z_f[:, mc, hv * 512:(hv + 1) * 512], ps[:],
                             AF.Identity, bias=b_in_c[:, mc:mc + 1], scale=SCL)

    def evac_sg(mc, hv, ps):
        nc.scalar.activation(sg_f[:, mc, hv * 512:(hv + 1) * 512], ps[:],
                             AF.Sigmoid, bias=b_a_c[:, mc:mc + 1], scale=SCL)

    def evac_g(mc, hv, ps):
        # silu(v) = v * sigmoid(v), v = psum/WS + b_g  (no Silu LUT on trn2)
        ps4 = ps[:].rearrange("p (b h w) -> p b h w", b=2, h=H, w=W)
        vt = scr.tile([128, 512], bf16, tag="gv", name=f"gv{mc}{hv}")
        nc.scalar.activation(vt[:], ps[:], AF.Identity,
                             bias=b_g_c[:, mc:mc + 1], scale=SCL)
        vt4 = vt[:].rearrange("p (b h w) -> p b h w", b=2, h=H, w=W)
        for bb in range(2):
            nc.scalar.activation(
                pad2(g_p, mc)[:, 2 * hv + bb, 2:2 + H, 2:2 + W], ps4[:, bb],
                AF.Sigmoid, bias=b_g_c[:, mc:mc + 1], scale=SCL)
            nc.vector.tensor_tensor(
                pad2(g_p, mc)[:, 2 * hv + bb, 2:2 + H, 2:2 + W],
                pad2(g_p, mc)[:, 2 * hv + bb, 2:2 + H, 2:2 + W],
                vt4[:, bb], op=OP.mult)

    mm_c(evac_z, w_in8t)
    mm_c(evac_g, w_g8t)
    mm_c(evac_sg, w_a8t)

    # ---------------- E..I: half-batch pipelined middle section ----------
    taps5 = [(i, j) for i in range(5) for j in range(5)]
    a_f = gt_f  # per-hv slices of gt_f are re-used as a = rho*sigma

    # q broadcast (only needs LN1 sums; emit early)
    z32 = pool.tile([128, T], fp32, name="z32")
    nc.vector.memset(z32[:], 0.0)
    q_col = pool.tile([128, NCC, BL], fp32, name="q_col")
    for kc in range(NCC):
        nc.vector.tensor_scalar(
            q_col[:, kc, :], sview[:, 0, :, kc], 1.0 / float(HWN), None,
            op0=OP.mult)
        for b in range(BL):
            nc.vector.tensor_scalar(
                qt[:, kc, b, :], z32[:], q_col[:, kc, b:b + 1], None, op0=OP.add)

    # --- DW5^T(g) for both halves (keeps PE busy while DVE runs ladders) ---
    for hv in range(HV):
        for kc in range(NCC):
            ps = pp_mm.tile([128, 512], fp32, tag="mm", name=f"cgt{kc}{hv}")
            for ti, (i, j) in enumerate(taps5):
                fi = (4 - i) * 5 + (4 - j)          # flipped kernel index
                rhs = pad2(g_p, kc)[:, 2 * hv:2 * hv + 2, i:i + H, j:j + W]
                nc.tensor.matmul(
                    ps[:], dsp_t[:, kc, fi, :], rhs,
                    start=(ti == 0), stop=(ti == 24))
            nc.scalar.copy(gt_f[:, kc, hv * 512:(hv + 1) * 512], ps[:])
        # gbar = raw sum_hw g on ScalarE
        for kc in range(NCC):
            for b in range(2 * hv, 2 * hv + 2):
                gsc = scr.tile([128, HWN], bf16, tag="st_scr", name=f"gb{kc}{b}")
                nc.scalar.activation(
                    gsc[:].rearrange("p (h w) -> p h w", h=H),
                    int2(g_p, kc)[:, b], AF.Copy,
                    accum_out=gbar_c[:, kc, b:b + 1])

    def emit_seed_ladder(hv):
        hsl = slice(hv * 512, (hv + 1) * 512)
        bs = range(2 * hv, 2 * hv + 2)
        # P = z*Gt with fused s0 accumulation
        for kc in range(NCC):
            for b in bs:
                sl = slice(b * HWN, (b + 1) * HWN)
                nc.vector.scalar_tensor_tensor(
                    u_f[:, kc, sl], z_f[:, kc, sl], 1.0, gt_f[:, kc, sl],
                    op0=OP.bypass, op1=OP.mult, accum_out=s0_c[:, kc, b:b + 1])
        # a = rho*sigma into the (now dead) Gt slice
        for kc in range(NCC):
            nc.vector.tensor_scalar(a_f[:, kc, hsl], sg_f[:, kc, hsl], RHO,
                                    None, op0=OP.mult)
        # Q-ladder: kc 0/1 fused STT+accum on DVE, kc 2 via GpSimd TT +
        # ScalarE accum
        cur, nxt = u_f, q2
        for t in range(T):
            for kc in range(2):
                for b in bs:
                    sl = slice(b * HWN, (b + 1) * HWN)
                    nc.vector.scalar_tensor_tensor(
                        nxt[:, kc, sl], cur[:, kc, sl], RHO, sg_f[:, kc, sl],
                        op0=OP.mult, op1=OP.mult,
                        accum_out=st_all[:, kc, b, t:t + 1])
            nc.gpsimd.tensor_tensor(nxt[:, 2, hsl], cur[:, 2, hsl],
                                    a_f[:, 2, hsl], op=OP.mult)
            for b in bs:
                j_sc = scr.tile([128, HWN], bf16, tag="st_scr", name=f"st{t}{b}")
                nc.scalar.activation(
                    j_sc[:], nxt[:, 2, b * HWN:(b + 1) * HWN], AF.Copy,
                    accum_out=st_all[:, 2, b, t:t + 1])
            cur, nxt = nxt, cur

    def emit_gate(hv):
        bs = slice(2 * hv, 2 * hv + 2)
        wsl = slice(hv * 2 * T, (hv + 1) * 2 * T)
        inv = 1.0 / float(HWN)
        for kc in range(NCC):
            nc.vector.scalar_tensor_tensor(
                s0gb[:, kc, bs], gbar_c[:, kc, bs], b_sp_c[:, kc:kc + 1],
                s0_c[:, kc, bs], op0=OP.mult, op1=OP.add)
            nc.vector.tensor_scalar(
                s0gb[:, kc, bs], s0gb[:, kc, bs], inv, None, op0=OP.mult)
            for t in range(T):
                nc.vector.scalar_tensor_tensor(
                    kv[:, kc, bs, t], st_all[:, kc, bs, t], -inv,
                    s0gb[:, kc, bs], op0=OP.mult, op1=OP.add)
        # k through W_out (bf16 path)
        for mc in range(NCC):
            ps = pp_sm.tile([128, 2 * T], fp32, tag="sm", name=f"kwm{mc}{hv}")
            for kc in range(NCC):
                nc.tensor.matmul(
                    ps[:], w_out_t[:, kc, mc * 128:(mc + 1) * 128],
                    kv[:, kc, bs, :], start=(kc == 0), stop=(kc == NCC - 1))
            nc.vector.tensor_scalar(
                kw[:, mc, wsl], ps[:], b_out_c[:, mc:mc + 1], None, op0=OP.add)
        # gate hidden
        psg = pp_sm.tile([GH, 2 * T], fp32, tag="sm", name=f"psg{hv}")
        for i in range(2 * NCC):
            rhs = qt[:, i, bs, :] if i < NCC else kw[:, i - NCC, wsl]
            nc.tensor.matmul(psg[:], wg1_t[:, i, :], rhs,
                             start=(i == 0), stop=(i == 2 * NCC - 1))
        nc.scalar.activation(hg[:, wsl], psg[:], AF.Gelu_apprx_tanh,
                             bias=bg1_c[:])
        psl = pp_sm.tile([1, 2 * T], fp32, tag="sm", name=f"psl{hv}")
        nc.tensor.matmul(psl[:], wg2_t[:], hg[:, wsl], start=True, stop=True)
        nc.vector.scalar_tensor_tensor(
            logits[:, wsl], psl[:], bg2_c[:], prior_r[:, wsl],
            op0=OP.add, op1=OP.add)
        # softmax over t
        lv = logits[:, wsl].rearrange("p (b t) -> p b t", b=2)
        nc.vector.tensor_reduce(mx_r[:, bs], lv, axis=AX.X, op=OP.max)
        for b in range(2 * hv, 2 * hv + 2):
            nc.vector.tensor_scalar(
                esh[:, b * T:(b + 1) * T], logits[:, b * T:(b + 1) * T],
                mx_r[:, b:b + 1], None, op0=OP.subtract)
        nc.scalar.activation(esh[:, wsl], esh[:, wsl], AF.Exp)
        nc.vector.tensor_reduce(
            se_r[:, bs], esh[:, wsl].rearrange("p (b t) -> p b t", b=2),
            axis=AX.X, op=OP.add)
        nc.vector.reciprocal(se_r[:, bs], se_r[:, bs])
        for b in range(2 * hv, 2 * hv + 2):
            nc.vector.tensor_scalar(
                wneg[:, b * T:(b + 1) * T], esh[:, b * T:(b + 1) * T],
                se_r[:, b:b + 1], -XOS, op0=OP.mult, op1=OP.mult)
        nc.gpsimd.partition_broadcast(wbc[:, wsl], wneg[:, wsl], channels=128)

    def emit_horner(hv):
        # Estrin in y = a^2:  -XOS*W = a*(d0 + d1*y + d2*y^2 + d3*y^3) with
        # d_k = -XOS*(w_{2k} + w_{2k+1}*a) built on ScalarE (per-partition
        # scale/bias APs), then XOS*F = z*(XOS + S).
        acc = u_f  # ladder buffers are dead (T even -> cur == u_f)
        hsl = slice(hv * 512, (hv + 1) * 512)
        for kc in range(NCC):
            ysc = scr.tile([128, 512], bf16, tag="ysq", name=f"y{kc}{hv}")
            nc.vector.tensor_tensor(ysc[:], a_f[:, kc, hsl], a_f[:, kc, hsl],
                                    op=OP.mult)
            dsc = scr.tile([128, 4, 512], bf16, tag="dply", name=f"d{kc}{hv}")
            for k in range(4):
                for bb in range(2):
                    b = 2 * hv + bb
                    nc.scalar.activation(
                        dsc[:, k, bb * HWN:(bb + 1) * HWN],
                        a_f[:, kc, b * HWN:(b + 1) * HWN], AF.Identity,
                        bias=wbc[:, b * T + 2 * k:b * T + 2 * k + 1],
                        scale=wbc[:, b * T + 2 * k + 1:b * T + 2 * k + 2])
            S = acc[:, kc, hsl]
            nc.vector.tensor_tensor(S, dsc[:, 3, :], ysc[:], op=OP.mult)
            nc.vector.tensor_tensor(S, S, dsc[:, 2, :], op=OP.add)
            nc.vector.tensor_tensor(S, S, ysc[:], op=OP.mult)
            nc.vector.tensor_tensor(S, S, dsc[:, 1, :], op=OP.add)
            nc.vector.tensor_tensor(S, S, ysc[:], op=OP.mult)
            nc.vector.tensor_tensor(S, S, dsc[:, 0, :], op=OP.add)
            nc.vector.tensor_tensor(S, S, a_f[:, kc, hsl], op=OP.mult)
            for bb in range(2):
                b = 2 * hv + bb
                nc.vector.scalar_tensor_tensor(
                    int2(f_p, kc)[:, b],
                    acc[:, kc, b * HWN:(b + 1) * HWN].rearrange(
                        "p (h w) -> p h w", h=H), XOS,
                    dense(z_f, kc)[:, b], op0=OP.add, op1=OP.mult)

    xo_rhs = xpos  # reuse xpos tile (fp8) as W_out rhs buffer; holds XOS*xo
    dw5f_ps = {}

    def emit_dw5f(hv):
        for kc in range(NCC):
            ps = pp_mm.tile([128, 512], fp32, tag="mm", name=f"cf{kc}{hv}")
            for ti, (i, j) in enumerate(taps5):
                rhs = pad2(f_p, kc)[:, 2 * hv:2 * hv + 2, i:i + H, j:j + W]
                nc.tensor.matmul(
                    ps[:], dsp_t[:, kc, ti, :], rhs,
                    start=(ti == 0), stop=(ti == 24))
            dw5f_ps[(kc, hv)] = ps

    def emit_xo_wout(hv):
        for kc in range(NCC):
            ps = dw5f_ps[(kc, hv)]
            ps4 = ps[:].rearrange("p (b h w) -> p b h w", b=2, h=H, w=W)
            for bb in range(2):
                b = 2 * hv + bb
                nc.vector.scalar_tensor_tensor(
                    dense(xo_rhs, kc)[:, b], ps4[:, bb], b_sp16_c[:, kc:kc + 1],
                    int2(g_p, kc)[:, b],
                    op0=OP.add, op1=OP.mult)
        for mc in range(NCC):
            ps = pp_mm.tile([128, 512], fp32, tag="mm", name=f"wo{mc}{hv}")
            nc.tensor.matmul(
                ps[:], w_out8t[:, 0:2, mc * 128:(mc + 1) * 128],
                xo_rhs[:, 0:2, hv * 512:(hv + 1) * 512],
                start=True, stop=False, perf_mode=DR)
            nc.tensor.matmul(
                ps[:], w_out8t[:, 2, mc * 128:(mc + 1) * 128],
                xo_rhs[:, 2, hv * 512:(hv + 1) * 512],
                start=False, stop=True)
            # psum holds WS*(x_out - b_out); rescale on ScalarE then add x
            xot = scr.tile([128, 512], bf16, tag="xot", name=f"xot{mc}{hv}")
            nc.scalar.activation(xot[:], ps[:], AF.Identity,
                                 bias=b_out_c[:, mc:mc + 1], scale=SCL)
            nc.vector.tensor_tensor(
                out1[:, mc, hv * 512:(hv + 1) * 512], xot[:],
                x_cm[:, mc, hv * 512:(hv + 1) * 512], op=OP.add)

    # pipeline: all DVE field work (ladder/gate/Horner per half) is emitted
    # before the PSUM evacuations so the in-order DVE queue never parks
    # behind an evac that waits on PE convolutions.
    emit_seed_ladder(0)
    emit_gate(0)
    emit_horner(0)
    emit_seed_ladder(1)
    emit_gate(1)
    emit_horner(1)
    emit_dw5f(0)
    emit_dw5f(1)
    emit_xo_wout(0)
    emit_xo_wout(1)

    # ---------------- J: LN2 ----------------
    o1b = q2  # bf16 copy of out1 (ladder buffer is dead)
    for kc in range(NCC):
        nc.scalar.copy(o1b[:, kc, :], out1[:, kc, :])
        nc.vector.tensor_tensor(u_f[:, kc, :], o1b[:, kc, :], o1b[:, kc, :],
                                op=OP.mult)   # squares into u_f
    for hv in range(HV):
        ps0 = pp_sm.tile([1, 512], fp32, tag="sm", name=f"l2s{hv}")
        for kc in range(NCC):
            nc.tensor.matmul(ps0[:], ones_c[:], o1b[:, kc, hv * 512:(hv + 1) * 512],
                             start=(kc == 0), stop=(kc == NCC - 1))
        nc.scalar.copy(stats2[:, 0, hv * 512:(hv + 1) * 512], ps0[:])
        ps1 = pp_sm.tile([1, 512], fp32, tag="sm", name=f"l2q{hv}")
        for kc in range(NCC):
            nc.tensor.matmul(ps1[:], ones_c[:], u_f[:, kc, hv * 512:(hv + 1) * 512],
                             start=(kc == 0), stop=(kc == NCC - 1))
        nc.scalar.copy(stats2[:, 1, hv * 512:(hv + 1) * 512], ps1[:])
    nc.scalar.mul(stats2[:, 0, :], stats2[:, 0, :], 1.0 / float(C))   # mu
    nc.scalar.mul(stats2[:, 1, :], stats2[:, 1, :], 1.0 / float(C))   # E[x^2]
    nc.vector.tensor_tensor(work2[:], stats2[:, 0, :], stats2[:, 0, :], op=OP.mult)
    nc.vector.tensor_tensor(work2[:], stats2[:, 1, :], work2[:], op=OP.subtract)
    nc.vector.tensor_scalar(work2[:], work2[:], EPS, None, op0=OP.add)
    # rstd = exp(-0.5*ln(var)) on ScalarE (avoids slow 1-partition DVE recip)
    nc.scalar.activation(lnv2[:], work2[:], AF.Ln)
    nc.scalar.activation(work2[:], lnv2[:], AF.Exp, scale=-0.5)
    nc.vector.tensor_copy(rhsS[:], work2[:])
    nc.vector.tensor_tensor(stats2[:, 0, :], stats2[:, 0, :], work2[:], op=OP.mult)
    nc.vector.tensor_scalar(stats2[:, 0, :], stats2[:, 0, :], -1.0, None,
                            op0=OP.mult)
    nc.vector.tensor_copy(rhsM[:], stats2[:, 0, :])
    nc.gpsimd.partition_broadcast(rstdB[:], rhsS[:], channels=128)
    nc.gpsimd.partition_broadcast(mrstdB[:], rhsM[:], channels=128)
    for kc in range(NCC):
        for hv in range(HV):
            hsl = slice(hv * 512, (hv + 1) * 512)
            t1 = scr.tile([128, 512], bf16, tag="ysq", name=f"lnt{kc}{hv}")
            nc.vector.tensor_tensor(t1[:], o1b[:, kc, hsl], rstdB[:, hsl],
                                    op=OP.mult)
            nc.vector.tensor_tensor(t1[:], t1[:], mrstdB[:, hsl], op=OP.add)
            nc.vector.tensor_scalar(
                yn8[:, kc, hsl], t1[:], g2c[:, kc:kc + 1],
                be2_c[:, kc:kc + 1], op0=OP.mult, op1=OP.add)

    # ---------------- K: MLP (fp8 DoubleRow) ----------------
    for jc in range(NHC):
        for hv in range(HV):
            ps = pp_mm.tile([128, 512], fp32, tag="mm", name=f"w1_{jc}{hv}")
            nc.tensor.matmul(
                ps[:], w1_8t[:, 0:2, jc * 128:(jc + 1) * 128],
                yn8[:, 0:2, hv * 512:(hv + 1) * 512],
                start=True, stop=False, perf_mode=DR)
            nc.tensor.matmul(
                ps[:], w1_8t[:, 2, jc * 128:(jc + 1) * 128],
                yn8[:, 2, hv * 512:(hv + 1) * 512],
                start=False, stop=True)
            ps4 = ps[:].rearrange("p (b h w) -> p b h w", b=2, h=H, w=W)
            for bb in range(2):
                nc.scalar.activation(
                    pad1(h1p, jc)[:, 2 * hv + bb, 1:1 + H, 1:1 + W], ps4[:, bb],
                    AF.Identity, bias=b1_c[:, jc:jc + 1], scale=SCL)
    taps3 = [(i, j) for i in range(3) for j in range(3)]
    for jc in range(NHC):
        ddw_t = dpool.tile([128, 9, 128], f8, tag="ddw", name=f"ddw{jc}")
        nc.sync.dma_start(ddw_t[:], d["ddw"][:, jc])
        for hv in range(HV):
            ps = pp_mm.tile([128, 512], fp32, tag="mm", name=f"cdw{jc}{hv}")
            for ti, (i, j) in enumerate(taps3):
                rhs = pad1(h1p, jc)[:, 2 * hv:2 * hv + 2, i:i + H, j:j + W]
                nc.tensor.matmul(ps[:], ddw_t[:, ti, :], rhs,
                                 start=(ti == 0), stop=(ti == 8))
            nc.scalar.activation(
                h1g[:, jc, hv * 512:(hv + 1) * 512], ps[:],
                AF.Gelu_apprx_tanh, bias=bdw_c[:, jc:jc + 1], scale=SCL)
    for mc in range(NCC):
        for hv in range(HV):
            ps = pp_mm.tile([128, 512], fp32, tag="mm", name=f"w2_{mc}{hv}")
            for jp in range(NHC // 2):
                nc.tensor.matmul(
                    ps[:],
                    w2_8t[:, 2 * jp:2 * jp + 2, mc * 128:(mc + 1) * 128],
                    h1g[:, 2 * jp:2 * jp + 2, hv * 512:(hv + 1) * 512],
                    start=(jp == 0), stop=(jp == NHC // 2 - 1), perf_mode=DR)
            mot = scr.tile([128, 512], bf16, tag="xot", name=f"mot{mc}{hv}")
            nc.scalar.activation(mot[:], ps[:], AF.Identity,
                                 bias=b2_c[:, mc:mc + 1], scale=SCL)
            nc.vector.tensor_tensor(
                out1[:, mc, hv * 512:(hv + 1) * 512],
                out1[:, mc, hv * 512:(hv + 1) * 512], mot[:], op=OP.add)
        nc.sync.dma_start(out_d[:, mc, :], out1[:, mc, :])

    ctx.close()


# ------------------------------------------------------------------
# host side
# ------------------------------------------------------------------

def _diagify(k2d, nchunks, add_identity=False):
    """k2d: (KH, KW, 1, Cn) -> (KH*KW, nchunks, 128, 128) bf16 diagonals."""
    kh, kw = k2d.shape[0], k2d.shape[1]
    out = np.zeros((kh * kw, nchunks, 128, 128), dtype=BF16)
    idx = np.arange(128)
    for t in range(kh * kw):
        vals = k2d[t // kw, t % kw, 0].astype(np.float32)
        if add_identity and t == (kh * kw) // 2:
            vals = vals + 1.0
        for c in range(nchunks):
            out[t, c, idx, idx] = vals[c * 128:(c + 1) * 128].astype(BF16)
    return out


def _f8(a):
    return np.clip(np.asarray(a, np.float32) * WS, -240.0, 240.0).astype(FP8)


def _prep_shared(w):
    """Build the shared (weight) input map from the raw input dict."""
    f32 = np.float32
    m = {}
    def pm(a):  # [k,128,...] -> [128,k,...] contiguous
        return np.ascontiguousarray(np.moveaxis(a, 1, 0))

    m["w_in8"] = _f8(pm(w["W_in"].astype(f32).reshape(NCC, 128, C)))
    m["w_a8"] = _f8(pm(w["W_a"].astype(f32).reshape(NCC, 128, C)))
    m["w_g8"] = _f8(pm(w["W_g"].astype(f32).reshape(NCC, 128, C)))
    wo = pm(w["W_out"].astype(f32).reshape(NCC, 128, C))
    m["w_out"] = wo.astype(BF16)
    # W_out fp8 carries WS/XOS so psum = WS * (W_out @ xo)
    m["w_out8"] = np.clip(wo * (WS / XOS), -240.0, 240.0).astype(FP8)
    m["w1_8"] = _f8(pm(w["W1"].astype(f32).reshape(NCC, 128, HID)))
    m["w2_8"] = _f8(pm(w["W2"].astype(f32).reshape(NHC, 128, C)))
    m["wg1"] = pm(w["Wg1"].astype(f32).reshape(2 * NCC, 128, GH)).astype(BF16)
    m["wg2"] = w["Wg2"].astype(f32).reshape(GH, 1).astype(BF16)
    m["dpos"] = np.ascontiguousarray(
        _diagify(np.asarray(w["w_pos"]), NCC, add_identity=True)
        .transpose(2, 1, 0, 3))
    m["dsp"] = np.ascontiguousarray(
        _diagify(np.asarray(w["k_sp"]), NCC).transpose(2, 1, 0, 3))
    # dw3 diagonals for the HID conv: [128, NHC, 9, 128] fp8 (x WS)
    vals9 = np.asarray(w["wdw"], f32).reshape(9, NHC, 128)
    ddw = np.zeros((128, NHC, 9, 128), dtype=FP8)
    idx = np.arange(128)
    ddw[idx, :, :, idx] = np.clip(
        vals9.transpose(2, 1, 0) * WS, -240.0, 240.0).astype(FP8)
    m["ddw"] = np.ascontiguousarray(ddw)
    # sp1 = 1 + conv3(ones) per channel: [128, NCC, HWN]
    wp = np.asarray(w["w_pos"], f32)[:, :, 0, :]           # (3,3,C)
    ones_im = np.ones((H, W), f32)
    s_acc = np.zeros((C, H, W), f32)
    for i in range(3):
        for j in range(3):
            shifted = np.zeros((H, W), f32)
            ys = slice(max(0, 1 - i), min(H, H + 1 - i))
            xs = slice(max(0, 1 - j), min(W, W + 1 - j))
            ys_s = slice(max(0, i - 1), min(H, H + i - 1))
            xs_s = slice(max(0, j - 1), min(W, W + j - 1))
            shifted[ys, xs] = ones_im[ys_s, xs_s]
            s_acc += wp[i, j][:, None, None] * shifted[None]
    sp1 = 1.0 + s_acc                                       # (C, H, W)
    m["sp1"] = np.ascontiguousarray(
        sp1.reshape(NCC, 128, HWN).transpose(1, 0, 2)).astype(BF16)
    for src, dst, n in [("b_in", "b_in", NCC), ("b_a", "b_a", NCC),
                        ("b_g", "b_g", NCC), ("b_sp", "b_sp", NCC),
                        ("b_out", "b_out", NCC), ("b2", "b2", NCC),
                        ("gamma1", "gamma1", NCC), ("beta1", "beta1", NCC),
                        ("b1", "b1", NHC), ("bdw", "bdw", NHC)]:
        m[dst] = np.ascontiguousarray(np.asarray(w[src], f32).reshape(n, 128).T)
    m["b_sp16"] = np.ascontiguousarray(
        (np.asarray(w["b_sp"], f32) * XOS).reshape(NCC, 128).T)
    m["b_pos"] = np.ascontiguousarray(
        np.asarray(w["b_pos"], f32).reshape(NCC, 128).T)
    m["g2c"] = np.ascontiguousarray(
        np.asarray(w["gamma2"], f32).reshape(NCC, 128).T)
    m["be2"] = np.ascontiguousarray(
        np.asarray(w["beta2"], f32).reshape(NCC, 128).T)
    m["bg1"] = np.asarray(w["bg1"], f32).reshape(GH, 1)
    m["bg2"] = np.asarray(w["bg2"], f32).reshape(1, 1)
    prior = np.zeros((T,), f32)
    prior[-1] = 4.0
    m["prior"] = np.tile(prior, BL)[None, :]
    return m


TRACE = False       # set True (e.g. from test.py) to capture an NTFF profile
LAST_RES = None


def kernel(**inputs):
    global _PROG, LAST_RES
    from concourse.bass_utils import run_bass_kernel_spmd

    if _PROG is None:
        _PROG = _build_program()
    nc = _PROG

    shared = _prep_shared(inputs)
    x = np.asarray(inputs["x"], np.float32)
    in_maps = []
    for i in range(NCORES):
        im = dict(shared)
        xs = x[i * BL:(i + 1) * BL].reshape(NTOK, C)
        # channel-major bf16 spine [128, NCC, NTOK]
        xcm = xs.T.reshape(NCC, 128, NTOK).transpose(1, 0, 2)
        im["x_cm"] = np.ascontiguousarray(xcm).astype(BF16)
        # zero-padded bf16 copy for the 3x3 positional conv
        xb = xs.astype(BF16).reshape(BL, H, W, C)
        xp = np.zeros((BL, H1, W1P, C), BF16)
        xp[:, 1:1 + H, 1:1 + W, :] = xb
        # -> [128, NCC, BL*H1*W1P]
        xp = xp.reshape(BL * H1 * W1P, C).T.reshape(NCC, 128, F1)
        im["x_pad"] = np.ascontiguousarray(xp.transpose(1, 0, 2))
        in_maps.append(im)

    res = run_bass_kernel_spmd(nc, in_maps, core_ids=list(range(NCORES)),
                               trace=TRACE)
    LAST_RES = res
    outs = []
    for r in res.results:
        o = r["out"]                      # [128, NCC, NTOK]
        o = o.transpose(1, 0, 2).reshape(C, NTOK).T    # [NTOK, C]
        outs.append(o.reshape(BL, H, W, C))
    return np.concatenate(outs, axis=0)
